# revision 7
# baseline (speedup 1.0000x reference)
"""Trainium2 kernel for the nn_Circuit coupled-mode ODE problem.

Math: dA/dt = i*diag(omega + gamma*|A|^2) A + T2 A, integrated t in [0,2],
sampled at 200 points; A is (1024 batch, 64 modes) complex, padded with ones
for modes 48..63.  L = T2 + i*diag(omega) is constant.

Strang splitting, linear part exact via the host-precomputed matrix
exponential E = expm(L h), nonlinear part as a 2nd-order phase rotation
u = z*(1 - theta^2/2 + i*theta), theta = gamma*h*|A|^2:

    z_{k+1} = E u_k = E z_k + (-E P S) qt_k + (-E/2) pp_k
    qt = z*theta,  pp = qt*theta          (theta unsigned; P = re/im pair
                                           swap, S = diag(-1,+1,...) folded
                                           into the host-precomputed weights)

The device streams out the chain state z_k (k=1..199); the host applies the
exact constant map y_k = E(-h/2) z_k while unsharding (the t=0 output is
known exactly on the host).

Per step on device (per 64-column chain, 2 chains per core):
  ACT:  s2 = Square(z, scale=sqrt(gh))   PSUM -> SBUF bf16
        zs = Copy(z)                     PSUM -> SBUF f32 (matmul rhs + DMA)
  PE:   th = (I+P) @ s2                  -> PSUM f32
        z' = E@zs (f32) + Eq@qt + Eh@pp  -> PSUM (accumulation group)
  V:    qt = zs * th ; pp = qt * th      -> SBUF bf16
  DMA:  out[k] <- zs'

Sharding: pure data parallel, batch 1024 = 8 cores x 128; 2 chains x 64.
"""

import os
import numpy as np

MODES = 64
INPUT_MODES = 48
BATCH = 1024
EVAL_PTS = 200
EPS = 1e-8
N_CORES = 8
B_LOC = BATCH // N_CORES  # 128
NT = EVAL_PTS - 1  # 199 steps
DT = 2.0 / NT
NCH = 2               # chains per core
BC = B_LOC // NCH     # 64 columns per chain

_CACHE = {}


# ---------------------------------------------------------------------------
# host-side math
# ---------------------------------------------------------------------------

def _t2_like_reference(params, omega, kappa):
    """Reproduce the reference's float32 jax computation of T2 exactly."""
    import jax

    try:
        cpu = jax.devices("cpu")[0]
    except Exception:
        cpu = None

    import contextlib

    ctx = jax.default_device(cpu) if cpu is not None else contextlib.nullcontext()
    with ctx:
        import jax.numpy as jnp

        n = MODES
        p = jnp.asarray(params, dtype=jnp.float32)
        n_off = n * (n - 1) // 2
        iu = jnp.triu_indices(n, 1)
        off = p[:n_off] + 1j * p[n_off:2 * n_off]
        H = jnp.zeros((n, n), dtype=jnp.complex64).at[iu].set(off.astype(jnp.complex64))
        H = H + H.conj().T
        d = p[2 * n_off:]
        diag = jnp.concatenate([d, -jnp.sum(d, keepdims=True)])
        H = H + jnp.diag(diag.astype(jnp.complex64))
        U = jax.scipy.linalg.expm(1j * H)
        I = jnp.eye(n, dtype=jnp.complex64)
        M = U.T @ U
        mix = M @ jnp.linalg.inv(I - M + EPS * I)
        T2 = -jnp.asarray(kappa, dtype=jnp.float32) * (
            0.5 * jnp.eye(n, dtype=jnp.float32) + mix
        )
        T2_re = np.asarray(jnp.real(T2), dtype=np.float32)
        T2_im = np.asarray(jnp.imag(T2), dtype=np.float32)
    return T2_re, T2_im


def _expm(M):
    """Matrix exponential of a (diagonalizable) complex matrix via eig."""
    w, V = np.linalg.eig(M)
    return (V * np.exp(w)) @ np.linalg.inv(V)


def _big_il(C):
    """Complex (64,64) -> real (128,128) operator in the interleaved re/im basis."""
    A = np.zeros((2 * MODES, 2 * MODES), dtype=np.float64)
    Cr, Ci = C.real, C.imag
    A[0::2, 0::2] = Cr
    A[0::2, 1::2] = -Ci
    A[1::2, 0::2] = Ci
    A[1::2, 1::2] = Cr
    return A


def _host_precompute(A0, params, omega, kappa, nonlinearity):
    T2_re, T2_im = _t2_like_reference(params, omega, kappa)
    L = T2_re.astype(np.float64) + 1j * T2_im.astype(np.float64)
    L = L + 1j * np.diag(omega.astype(np.float64))

    E = _big_il(_expm(L * DT))          # full-step propagator E(h)
    A2m = _big_il(_expm(-L * (DT / 2)))  # host output map E(-h/2)
    perm = np.arange(128) ^ 1            # re/im pair swap
    sgn = np.tile([-1.0, 1.0], MODES)    # s_{2j}=-1, s_{2j+1}=+1

    import ml_dtypes

    # lhsT arrangements: matmul computes lhsT.T @ rhs
    wE = np.ascontiguousarray(E.T, dtype=np.float32)
    # z' += (-E P S) qt  ->  lhsT row p = -s_p * E.T[p^1]
    wEq = np.ascontiguousarray(
        -(sgn[:, None] * E.T[perm, :]), dtype=ml_dtypes.bfloat16)
    wEh = np.ascontiguousarray(-0.5 * E.T, dtype=ml_dtypes.bfloat16)
    wTh = np.ascontiguousarray(
        np.eye(128)[perm, :] + np.eye(128), dtype=ml_dtypes.bfloat16
    )  # (I+P), symmetric

    # initial state z_0 = E(h/2) y_0, interleaved mode-major (128, BATCH)
    y0 = np.zeros((2 * MODES, BATCH), dtype=np.float64)
    y0[0:2 * INPUT_MODES:2, :] = A0[:, :, 0].astype(np.float64).T
    y0[1:2 * INPUT_MODES:2, :] = A0[:, :, 1].astype(np.float64).T
    y0[2 * INPUT_MODES::2, :] = 1.0
    E2 = _big_il(_expm(L * (DT / 2)))
    z0 = (E2 @ y0).astype(np.float32)
    y0M = y0.astype(np.float32)

    gh = nonlinearity.astype(np.float64) * DT  # per-mode gamma*h > 0
    sc = np.sqrt(np.repeat(gh, 2)).astype(np.float32).reshape(128, 1)

    return dict(wE=wE, wEq=wEq, wEh=wEh, wTh=wTh, z0=z0, y0M=y0M, sc=sc,
                A2m=A2m.astype(np.float32))


# ---------------------------------------------------------------------------
# device kernel
# ---------------------------------------------------------------------------

def _build_nc():
    import concourse.bass as bass
    import concourse.bacc as bacc
    import concourse.tile as tile
    import concourse.mybir as mybir

    f32 = mybir.dt.float32
    f32r = mybir.dt.float32r
    bf16 = mybir.dt.bfloat16
    Square = mybir.ActivationFunctionType.Square
    Copy = mybir.ActivationFunctionType.Copy
    mult = mybir.AluOpType.mult
    P = 128
    use_f32r = os.environ.get("STATE_F32R", "0") == "1"

    nc = bacc.Bacc("TRN2", target_bir_lowering=False, debug=False,
                   num_devices=N_CORES)

    wE_d = nc.dram_tensor("wE", [P, P], f32, kind="ExternalInput").ap()
    wEq_d = nc.dram_tensor("wEq", [P, P], bf16, kind="ExternalInput").ap()
    wEh_d = nc.dram_tensor("wEh", [P, P], bf16, kind="ExternalInput").ap()
    wTh_d = nc.dram_tensor("wTh", [P, P], bf16, kind="ExternalInput").ap()
    z0_d = nc.dram_tensor("z0", [P, B_LOC], f32, kind="ExternalInput").ap()
    sc_d = nc.dram_tensor("sc", [P, 1], f32, kind="ExternalInput").ap()
    # z_1..z_199, chain-blocked: out[k][c] is the contiguous (128, BC) tile
    out_d = nc.dram_tensor("out", [NT, NCH, P, BC], f32,
                           kind="ExternalOutput").ap()

    with tile.TileContext(nc) as tc:
        with (
            tc.tile_pool(name="const", bufs=1) as cpool,
            tc.tile_pool(name="st", bufs=3) as spool,
            tc.tile_pool(name="nl", bufs=2) as npool,
            tc.tile_pool(name="pz", bufs=2, space="PSUM") as zpsum,
            tc.tile_pool(name="pt", bufs=2, space="PSUM") as tpsum,
        ):
            wE_t = cpool.tile([P, P], f32, tag="wE")
            wEq_t = cpool.tile([P, P], bf16, tag="wEq")
            wEh_t = cpool.tile([P, P], bf16, tag="wEh")
            wTh_t = cpool.tile([P, P], bf16, tag="wTh")
            sc_t = cpool.tile([P, 1], f32, tag="sc")
            z0_t = cpool.tile([P, B_LOC], f32, tag="z0")
            nc.sync.dma_start(wE_t[:], wE_d[:])
            nc.sync.dma_start(wEq_t[:], wEq_d[:])
            nc.sync.dma_start(wEh_t[:], wEh_d[:])
            nc.sync.dma_start(wTh_t[:], wTh_d[:])
            nc.sync.dma_start(sc_t[:], sc_d[:])
            nc.sync.dma_start(z0_t[:], z0_d[:])

            # per-chain current state APs (SBUF) and PSUM z (None at k=0)
            zs_cur = [z0_t[:, c * BC:(c + 1) * BC] for c in range(NCH)]
            zp_cur = [None, None]

            def step(k, c):
                zsrc = zp_cur[c][:] if zp_cur[c] is not None else zs_cur[c]
                s2 = npool.tile([P, BC], bf16, tag=f"s2{c}")
                nc.scalar.activation(s2[:], zsrc, Square, scale=sc_t[:])
                th = tpsum.tile([P, BC], f32, tag=f"th{c}")
                nc.tensor.matmul(th[:], wTh_t[:], s2[:], start=True, stop=True)
                qt = npool.tile([P, BC], bf16, tag=f"qt{c}")
                nc.vector.tensor_tensor(qt[:], zs_cur[c], th[:], mult)
                pp = npool.tile([P, BC], bf16, tag=f"pp{c}")
                nc.vector.tensor_tensor(pp[:], qt[:], th[:], mult)

                zn = zpsum.tile([P, BC], f32, tag=f"z{c}")
                if use_f32r:
                    nc.tensor.matmul(zn[:], wE_t[:].bitcast(f32r),
                                     zs_cur[c].bitcast(f32r),
                                     start=True, stop=False)
                else:
                    nc.tensor.matmul(zn[:], wE_t[:], zs_cur[c], start=True,
                                     stop=False)
                nc.tensor.matmul(zn[:], wEq_t[:], qt[:], start=False,
                                 stop=False)
                nc.tensor.matmul(zn[:], wEh_t[:], pp[:], start=False,
                                 stop=True)
                zs = spool.tile([P, BC], f32, tag=f"zs{c}")
                # balance PSUM evacuations: chain 0 on ScalarE, chain 1 on DVE
                if c == 0:
                    nc.scalar.activation(zs[:], zn[:], Copy)
                else:
                    nc.vector.tensor_copy(zs[:], zn[:])
                nc.sync.dma_start(out_d[k, c], zs[:])
                zs_cur[c] = zs[:]
                zp_cur[c] = zn

            # chain 1 runs one step behind chain 0 in program order, so each
            # engine alternates between the chains' serial phases and the two
            # critical paths overlap.
            for k in range(NT + 1):
                if k < NT:
                    step(k, 0)
                if k > 0:
                    step(k - 1, 1)

    nc.compile()
    return nc


def _get_compiled():
    if "nc" not in _CACHE:
        _CACHE["nc"] = _build_nc()
    return _CACHE["nc"]


def _run(host, trace=False, tmpdir=None):
    from concourse.bass_utils import run_bass_kernel_spmd

    nc = _get_compiled()
    in_maps = []
    for i in range(N_CORES):
        sl = slice(i * B_LOC, (i + 1) * B_LOC)
        in_maps.append({
            "wE": host["wE"],
            "wEq": host["wEq"],
            "wEh": host["wEh"],
            "wTh": host["wTh"],
            "z0": np.ascontiguousarray(host["z0"][:, sl]),
            "sc": host["sc"],
        })
    res = run_bass_kernel_spmd(nc, in_maps, list(range(N_CORES)), trace=trace,
                               tmpdir=tmpdir)

    A2m = host["A2m"]
    full = np.empty((EVAL_PTS, BATCH, MODES, 2), dtype=np.float32)
    # t = 0: exact initial state
    full[0] = host["y0M"].T.reshape(BATCH, MODES, 2)
    for i in range(N_CORES):
        sl = slice(i * B_LOC, (i + 1) * B_LOC)
        arr = res.results[i]["out"]  # (NT, NCH, 128, BC)
        z = arr.transpose(0, 2, 1, 3).reshape(NT, 128, B_LOC)
        y = np.matmul(A2m, z)  # exact map y_k = E(-h/2) z_k
        full[1:, sl] = y.transpose(0, 2, 1).reshape(NT, B_LOC, MODES, 2)
    return full, res


def kernel(A0, params, omega, kappa, nonlinearity):
    A0 = np.asarray(A0, dtype=np.float32)
    params = np.asarray(params, dtype=np.float32)
    omega = np.asarray(omega, dtype=np.float32)
    kappa = np.asarray(kappa, dtype=np.float32)
    nonlinearity = np.asarray(nonlinearity, dtype=np.float32)

    host = _host_precompute(A0, params, omega, kappa, nonlinearity)
    full, _ = _run(host, trace=False)
    return full


# revision 14
# speedup vs baseline: 1.6168x; 1.6168x over previous
"""Trainium2 kernel for the nn_Circuit coupled-mode ODE problem.

Math: dA/dt = i*diag(omega + gamma*|A|^2) A + T2 A, integrated t in [0,2],
sampled at 200 points; A is (1024 batch, 64 modes) complex, padded with ones
for modes 48..63.  L = T2 + i*diag(omega) is constant.

Strang splitting; linear part exact via E = expm(L h); nonlinear part as a
2nd-order phase rotation u = z*(1 - theta^2/2 + i*theta), theta =
gamma*h*|A|^2 (signs of the sin term folded into host weights):

    z_{k+1} = E z_k + (-E P S) qt_k + (-E/2) pp_k,  qt = z*theta, pp = z*theta^2

Device structure (one full-width 128-column chain per core; the serial
dependency cycle is the bottleneck, so theta is computed from the linearly
PREDICTED state E@z_k (error O(theta^2) per step, validated 1.5e-3):

  PSUM banks:  Z: zt_{k+1} = E@zs_k (f32, exact linear part)
               R: zr_{k+1} = E@zs_k in f32r (fast 1-pass copy for theta only)
               C: corr_{k+1} = Eq@qt_k + Eh@pp_k
               T: th_{k+1} = (I+P)@s2_{k+1}
  ACT:  s2 = Square(zr, scale=sqrt(gh));  t2 = Square(th)
  V:    zs_k = zt_k + corr_k (-> SBUF f32, also the DMA source)
        qt_k = zs_k * th_k;   pp_k = zr_k * t2_k
  Host: y_k = E(-h/2) z_k exactly, during unshard; y_0 known exactly.

Sharding: pure data parallel, batch 1024 = 8 cores x 128.
"""

import os
import numpy as np

MODES = 64
INPUT_MODES = 48
BATCH = 1024
EVAL_PTS = 200
EPS = 1e-8
N_CORES = 8
B_LOC = BATCH // N_CORES  # 128
NT = EVAL_PTS - 1  # 199 steps
DT = 2.0 / NT

_CACHE = {}


# ---------------------------------------------------------------------------
# host-side math
# ---------------------------------------------------------------------------

def _t2_like_reference(params, omega, kappa):
    """Reproduce the reference's float32 jax computation of T2 exactly."""
    import jax

    try:
        cpu = jax.devices("cpu")[0]
    except Exception:
        cpu = None

    import contextlib

    ctx = jax.default_device(cpu) if cpu is not None else contextlib.nullcontext()
    with ctx:
        import jax.numpy as jnp

        n = MODES
        p = jnp.asarray(params, dtype=jnp.float32)
        n_off = n * (n - 1) // 2
        iu = jnp.triu_indices(n, 1)
        off = p[:n_off] + 1j * p[n_off:2 * n_off]
        H = jnp.zeros((n, n), dtype=jnp.complex64).at[iu].set(off.astype(jnp.complex64))
        H = H + H.conj().T
        d = p[2 * n_off:]
        diag = jnp.concatenate([d, -jnp.sum(d, keepdims=True)])
        H = H + jnp.diag(diag.astype(jnp.complex64))
        U = jax.scipy.linalg.expm(1j * H)
        I = jnp.eye(n, dtype=jnp.complex64)
        M = U.T @ U
        mix = M @ jnp.linalg.inv(I - M + EPS * I)
        T2 = -jnp.asarray(kappa, dtype=jnp.float32) * (
            0.5 * jnp.eye(n, dtype=jnp.float32) + mix
        )
        T2_re = np.asarray(jnp.real(T2), dtype=np.float32)
        T2_im = np.asarray(jnp.imag(T2), dtype=np.float32)
    return T2_re, T2_im


def _expm(M):
    """Matrix exponential of a (diagonalizable) complex matrix via eig."""
    w, V = np.linalg.eig(M)
    return (V * np.exp(w)) @ np.linalg.inv(V)


def _big_il(C):
    """Complex (64,64) -> real (128,128) operator in the interleaved re/im basis."""
    A = np.zeros((2 * MODES, 2 * MODES), dtype=np.float64)
    Cr, Ci = C.real, C.imag
    A[0::2, 0::2] = Cr
    A[0::2, 1::2] = -Ci
    A[1::2, 0::2] = Ci
    A[1::2, 1::2] = Cr
    return A


def _host_precompute(A0, params, omega, kappa, nonlinearity):
    import ml_dtypes

    T2_re, T2_im = _t2_like_reference(params, omega, kappa)
    L = T2_re.astype(np.float64) + 1j * T2_im.astype(np.float64)
    L = L + 1j * np.diag(omega.astype(np.float64))

    E = _big_il(_expm(L * DT))          # full-step propagator E(h)
    A2m = _big_il(_expm(-L * (DT / 2)))  # host output map E(-h/2)
    perm = np.arange(128) ^ 1            # re/im pair swap
    sgn = np.tile([-1.0, 1.0], MODES)    # s_{2j}=-1, s_{2j+1}=+1

    # lhsT arrangements: matmul computes lhsT.T @ rhs
    wE = np.ascontiguousarray(E.T, dtype=np.float32)
    # z' += (-E P S) qt  ->  lhsT row p = -s_p * E.T[p^1]
    wEq = np.ascontiguousarray(
        -(sgn[:, None] * E.T[perm, :]), dtype=ml_dtypes.bfloat16)
    wEh = np.ascontiguousarray(-0.5 * E.T, dtype=ml_dtypes.bfloat16)
    wTh = np.ascontiguousarray(
        np.eye(128)[perm, :] + np.eye(128), dtype=ml_dtypes.bfloat16
    )  # (I+P), symmetric

    # initial state z_0 = E(h/2) y_0, interleaved mode-major (128, BATCH)
    y0 = np.zeros((2 * MODES, BATCH), dtype=np.float64)
    y0[0:2 * INPUT_MODES:2, :] = A0[:, :, 0].astype(np.float64).T
    y0[1:2 * INPUT_MODES:2, :] = A0[:, :, 1].astype(np.float64).T
    y0[2 * INPUT_MODES::2, :] = 1.0
    E2 = _big_il(_expm(L * (DT / 2)))
    z0 = (E2 @ y0).astype(np.float32)
    y0M = y0.astype(np.float32)

    gh = nonlinearity.astype(np.float64) * DT  # per-mode gamma*h > 0
    sc = np.sqrt(np.repeat(gh, 2)).astype(np.float32).reshape(128, 1)

    return dict(wE=wE, wEq=wEq, wEh=wEh, wTh=wTh, z0=z0, y0M=y0M, sc=sc,
                A2m=A2m.astype(np.float32))


# ---------------------------------------------------------------------------
# device kernel
# ---------------------------------------------------------------------------

def _build_nc():
    import concourse.bass as bass
    import concourse.bacc as bacc
    import concourse.tile as tile
    import concourse.mybir as mybir

    f32 = mybir.dt.float32
    f32r = mybir.dt.float32r
    bf16 = mybir.dt.bfloat16
    Square = mybir.ActivationFunctionType.Square
    mult = mybir.AluOpType.mult
    add = mybir.AluOpType.add
    P = 128
    N = B_LOC  # 128 columns, one full-width chain

    nc = bacc.Bacc("TRN2", target_bir_lowering=False, debug=False,
                   num_devices=N_CORES)

    wE_d = nc.dram_tensor("wE", [P, P], f32, kind="ExternalInput").ap()
    wEq_d = nc.dram_tensor("wEq", [P, P], bf16, kind="ExternalInput").ap()
    wEh_d = nc.dram_tensor("wEh", [P, P], bf16, kind="ExternalInput").ap()
    wTh_d = nc.dram_tensor("wTh", [P, P], bf16, kind="ExternalInput").ap()
    z0_d = nc.dram_tensor("z0", [P, N], f32, kind="ExternalInput").ap()
    sc_d = nc.dram_tensor("sc", [P, 1], f32, kind="ExternalInput").ap()
    out_d = nc.dram_tensor("out", [NT, P, N], f32, kind="ExternalOutput").ap()

    with tile.TileContext(nc) as tc:
        with (
            tc.tile_pool(name="const", bufs=1) as cpool,
            tc.tile_pool(name="st", bufs=3) as spool,
            tc.tile_pool(name="nl", bufs=2) as npool,
            tc.tile_pool(name="pz", bufs=2, space="PSUM") as zpsum,
            tc.tile_pool(name="pr", bufs=2, space="PSUM") as rpsum,
            tc.tile_pool(name="pt", bufs=2, space="PSUM") as tpsum,
        ):
            wE_t = cpool.tile([P, P], f32, tag="wE")
            wEr_t = cpool.tile([P, P], f32r, tag="wEr")
            wEq_t = cpool.tile([P, P], bf16, tag="wEq")
            wEh_t = cpool.tile([P, P], bf16, tag="wEh")
            wTh_t = cpool.tile([P, P], bf16, tag="wTh")
            sc_t = cpool.tile([P, 1], f32, tag="sc")
            z0_t = cpool.tile([P, N], f32, tag="z0")
            nc.sync.dma_start(wE_t[:], wE_d[:])
            nc.sync.dma_start(wEq_t[:], wEq_d[:])
            nc.sync.dma_start(wEh_t[:], wEh_d[:])
            nc.sync.dma_start(wTh_t[:], wTh_d[:])
            nc.sync.dma_start(sc_t[:], sc_d[:])
            nc.sync.dma_start(z0_t[:], z0_d[:])
            # f32r-rounded copies: the verifier requires f32r matmul inputs
            # to be produced as f32r
            nc.vector.tensor_copy(wEr_t[:], wE_t[:])
            z0r_t = cpool.tile([P, N], f32r, tag="z0r")
            nc.vector.tensor_copy(z0r_t[:], z0_t[:])

            # --- prologue: theta_0 / t2_0 / rotation terms from z_0 (SBUF) ---
            s2 = npool.tile([P, N], bf16, tag="s2")
            nc.scalar.activation(s2[:], z0_t[:], Square, scale=sc_t[:])
            th = tpsum.tile([P, N], f32, tag="th")
            nc.tensor.matmul(th[:], wTh_t[:], s2[:], start=True, stop=True)
            t2 = npool.tile([P, N], bf16, tag="t2")
            nc.scalar.activation(t2[:], th[:], Square)
            qt = npool.tile([P, N], bf16, tag="qt")
            nc.vector.tensor_tensor(qt[:], z0_t[:], th[:], mult)
            pp = npool.tile([P, N], bf16, tag="pp")
            nc.vector.tensor_tensor(pp[:], z0_t[:], t2[:], mult)

            zs = z0_t[:]
            zsr = z0r_t[:]
            use_f32_state = os.environ.get("STATE_F32", "0") == "1"

            for k in range(NT):
                # prediction bank (theta only) + state bank, same weights
                zrn = rpsum.tile([P, N], f32, tag="zr")
                nc.tensor.matmul(zrn[:], wEr_t[:], zsr, start=True, stop=True)
                zn = zpsum.tile([P, N], f32, tag="zn")
                if use_f32_state:
                    nc.tensor.matmul(zn[:], wE_t[:], zs, start=True,
                                     stop=False)
                else:
                    nc.tensor.matmul(zn[:], wEr_t[:], zsr, start=True,
                                     stop=False)
                nc.tensor.matmul(zn[:], wEq_t[:], qt[:], start=False,
                                 stop=False)
                nc.tensor.matmul(zn[:], wEh_t[:], pp[:], start=False,
                                 stop=True)

                # theta pipeline for step k+1 off the predicted state
                s2 = npool.tile([P, N], bf16, tag="s2")
                nc.scalar.activation(s2[:], zrn[:], Square, scale=sc_t[:])
                th = tpsum.tile([P, N], f32, tag="th")
                nc.tensor.matmul(th[:], wTh_t[:], s2[:], start=True, stop=True)
                t2 = npool.tile([P, N], bf16, tag="t2")
                nc.scalar.activation(t2[:], th[:], Square)

                # materialize z_{k+1} (f32r-rounded: it is the matmul rhs),
                # stream it out, build rotation terms
                zs_n = spool.tile([P, N], f32r, tag="zs")
                nc.vector.tensor_copy(zs_n[:], zn[:])
                nc.sync.dma_start(out_d[k], zs_n[:].bitcast(f32))
                if k < NT - 1:
                    qt = npool.tile([P, N], bf16, tag="qt")
                    nc.vector.tensor_tensor(qt[:], zs_n[:].bitcast(f32),
                                            th[:], mult)
                    pp = npool.tile([P, N], bf16, tag="pp")
                    nc.vector.tensor_tensor(pp[:], zrn[:], t2[:], mult)
                zs = zs_n[:].bitcast(f32)
                zsr = zs_n[:]

    nc.compile()
    return nc


def _get_compiled():
    if "nc" not in _CACHE:
        _CACHE["nc"] = _build_nc()
    return _CACHE["nc"]


def _run(host, trace=False, tmpdir=None):
    from concourse.bass_utils import run_bass_kernel_spmd

    nc = _get_compiled()
    in_maps = []
    for i in range(N_CORES):
        sl = slice(i * B_LOC, (i + 1) * B_LOC)
        in_maps.append({
            "wE": host["wE"],
            "wEq": host["wEq"],
            "wEh": host["wEh"],
            "wTh": host["wTh"],
            "z0": np.ascontiguousarray(host["z0"][:, sl]),
            "sc": host["sc"],
        })
    res = run_bass_kernel_spmd(nc, in_maps, list(range(N_CORES)), trace=trace,
                               tmpdir=tmpdir)

    A2m = host["A2m"]
    full = np.empty((EVAL_PTS, BATCH, MODES, 2), dtype=np.float32)
    full[0] = host["y0M"].T.reshape(BATCH, MODES, 2)
    for i in range(N_CORES):
        sl = slice(i * B_LOC, (i + 1) * B_LOC)
        z = res.results[i]["out"]  # (NT, 128, B_LOC)
        y = np.matmul(A2m, z)      # exact map y_k = E(-h/2) z_k
        full[1:, sl] = y.transpose(0, 2, 1).reshape(NT, B_LOC, MODES, 2)
    return full, res


def kernel(A0, params, omega, kappa, nonlinearity):
    A0 = np.asarray(A0, dtype=np.float32)
    params = np.asarray(params, dtype=np.float32)
    omega = np.asarray(omega, dtype=np.float32)
    kappa = np.asarray(kappa, dtype=np.float32)
    nonlinearity = np.asarray(nonlinearity, dtype=np.float32)

    host = _host_precompute(A0, params, omega, kappa, nonlinearity)
    full, _ = _run(host, trace=False)
    return full
